# revision 10
# baseline (speedup 1.0000x reference)
"""DistMult metapath scoring kernel for Trainium2 (8 NeuronCores).

Math (from the reference): every output group reduces to
    score = emb_h[idx] @ c        with c = K @ s a fixed [d] vector per group
where s is a sum of gathered embedding rows:
    pos0: idx=ei0[0]         s=sum emb_A[ei0[1]]     c=K0@s
    pos1: idx=ei1[0]         s=sum emb_B[ei1[1]]     c=K1@s
    nh0:  idx=nh0.flat       s=sum emb_A[nh0[:,0]]   c=16*K0@s
    nh1:  idx=nh1.flat       s=sum emb_A[nh1[:,0]]   c=16*K1@s
    nt0:  idx=nt0[:,0] (x16) s=sum emb_A[nt0.flat]   c=K0@s
    nt1:  idx=nt1[:,0] (x16) s=sum emb_B[nt1.flat]   c=K1@s

Bulk row gathers use InstDMAGatherAnt (dma_gather): thousands of rows per
instruction, int16 indices wrapped [16, n/16]. Tables are sharded into
25000-row shards (int16 range) with appended zero rows; indices are bucketed
by shard on the host and padded with the zero-row index (harmless for sums;
dot-phase pad results are dropped by the host inverse permutation).

Two SPMD launches on 8 cores:
  L1: bucketed dma_gather row sums, sharded over indices -> partials [6,128].
  L2: head (reduce partials, c = K@s) + bucketed dma_gather of score rows,
      on-device dot with c; nt groups use order-preserving per-partition
      indirect gathers + on-device x16 broadcast.
Host glue: index bucketing/padding (layout only), stacking of partials,
inverse-permutation unshard of the scores.
"""

import sys
from contextlib import ExitStack

import numpy as np

sys.path.insert(0, "/opt/trn_rl_repo")

import concourse.bass as bass
from concourse import bacc, mybir
from concourse.bass import IndirectOffsetOnAxis
from concourse.bass_utils import run_bass_kernel_spmd
from concourse.masks import make_identity
from concourse.tile import TileContext

D = 128
E = 50000
S = 16
NA = 100000
NB = 50000
NCORES = 8

EC = E // NCORES        # 6250 edge items per core
FC = (E * S) // NCORES  # 100000 flat neg items per core

SH = 25000              # table rows per shard
SHP = 25024             # shard rows incl. zero pad rows
ZIDX = 25000            # local index of a guaranteed-zero row
NSH_A, NSH_B = 4, 2

F32 = mybir.dt.float32
I32 = mybir.dt.int32
I16 = mybir.dt.int16
X = mybir.AxisListType.X
ADD = mybir.AluOpType.add

# chunk lists per bucket capacity (each chunk = one dma_gather instruction)
CH_2048 = [2048]
CH_4096 = [4096]
CH_26624 = [4096] * 6 + [2048]
CH_52224 = [4096] * 12 + [2048, 2048]

# L1 groups: (name, items/core, table, per-shard chunk list)
L1_GROUPS = [
    ("s0", EC, "A", CH_2048),
    ("s1", EC, "B", CH_4096),
    ("h0", EC, "A", CH_2048),
    ("h1", EC, "A", CH_2048),
    ("t0", FC, "A", CH_26624),
    ("t1", FC, "B", CH_52224),
]

# L2 bucketed segments: (name, items/core, table, c column, per-shard chunks)
L2_SEGS = [
    ("pos0", EC, "A", 0, CH_2048),
    ("pos1", EC, "A", 1, CH_2048),
    ("nh0", FC, "A", 2, CH_26624),
    ("nh1", FC, "A", 3, CH_26624),
]
# L2 ordered segments (x16 expand): (name, c column, table)
NT_SEGS = [("nt0", 4, "A"), ("nt1", 5, "B")]
NT_W = 49               # 6272 = 49*128 padded bases per core
NT_PAD = NT_W * 128


def _nsh(t):
    return NSH_A if t == "A" else NSH_B


def _cap(chunks):
    return sum(chunks)


def build_l1() -> bass.Bass:
    nc = bacc.Bacc(None, target_bir_lowering=False)
    tabA = nc.dram_tensor("tabA", [NSH_A, SHP, D], F32, kind="ExternalInput")
    tabB = nc.dram_tensor("tabB", [NSH_B, SHP, D], F32, kind="ExternalInput")
    idx_in = {}
    for name, L, t, chunks in L1_GROUPS:
        W = _cap(chunks) * _nsh(t) // 16
        idx_in[name] = nc.dram_tensor("x_" + name, [128, W], I16, kind="ExternalInput")
    out = nc.dram_tensor("partials", [6, D], F32, kind="ExternalOutput")

    with ExitStack() as ctx:
        tc = ctx.enter_context(TileContext(nc))
        sing = ctx.enter_context(tc.tile_pool(name="sing", bufs=1))
        gb = ctx.enter_context(tc.tile_pool(name="gbuf", bufs=4))
        ib = ctx.enter_context(tc.tile_pool(name="ibuf", bufs=2))
        accp = ctx.enter_context(tc.tile_pool(name="accp", bufs=1))
        pp = ctx.enter_context(tc.tile_pool(name="psum", bufs=2, space="PSUM"))

        ones = sing.tile([128, 1], F32)
        nc.vector.memset(ones, 1.0)
        red_all = accp.tile([128, 6 * D], F32)
        acc = accp.tile([128, 4096], F32)

        for g, (name, L, t, chunks) in enumerate(L1_GROUPS):
            tab = tabA if t == "A" else tabB
            nsh = _nsh(t)
            W = _cap(chunks) * nsh // 16
            it = ib.tile([128, W], I16, tag="idx")
            nc.sync.dma_start(out=it[:, :], in_=idx_in[name][:, :])
            maxw = max(chunks)
            first = True
            col = 0
            for s in range(nsh):
                for n in chunks:
                    wc = n // 16
                    bt = gb.tile([128, 4096], F32, tag="g")
                    nc.gpsimd.dma_gather(
                        out_ap=bt[:, :n].rearrange("p (c e) -> p c e", e=D),
                        in_ap=tab[s],
                        idxs_ap=it[:, col : col + wc],
                        num_idxs=n,
                        num_idxs_reg=n,
                        elem_size=D,
                        single_packet=False,
                    )
                    if first:
                        nc.vector.tensor_copy(acc[:, :n], bt[:, :n])
                        first = False
                    else:
                        nc.vector.tensor_add(
                            out=acc[:, :n], in0=acc[:, :n], in1=bt[:, :n]
                        )
                    col += wc
            red = red_all[:, g * D : (g + 1) * D]
            view = acc[:, :maxw].rearrange("p (j d) -> p d j", d=D)
            nc.vector.tensor_reduce(out=red, in_=view, axis=X, op=ADD)

        pvec = sing.tile([1, 6 * D], F32)
        for h in range(2):
            ps = pp.tile([1, 3 * D], F32, tag="ps")
            nc.tensor.matmul(
                out=ps[:, :],
                lhsT=ones[:, :],
                rhs=red_all[:, h * 3 * D : (h + 1) * 3 * D],
                start=True,
                stop=True,
            )
            nc.vector.tensor_copy(pvec[:, h * 3 * D : (h + 1) * 3 * D], ps[:, :])
        nc.sync.dma_start(
            out=out[:, :].rearrange("a b -> (a b)")[None, :], in_=pvec[:, :]
        )
    nc.compile()
    return nc


def build_l2() -> bass.Bass:
    nc = bacc.Bacc(None, target_bir_lowering=False)
    tabA = nc.dram_tensor("tabA", [NSH_A, SHP, D], F32, kind="ExternalInput")
    tabB = nc.dram_tensor("tabB", [NSH_B, SHP, D], F32, kind="ExternalInput")
    partials = nc.dram_tensor("partials", [8, 6 * D], F32, kind="ExternalInput")
    rel = nc.dram_tensor("rel", [2, D, D], F32, kind="ExternalInput")
    ins, outs = {}, {}
    for name, L, t, cc, chunks in L2_SEGS:
        cap = _cap(chunks) * _nsh(t)
        ins[name] = nc.dram_tensor(
            "x_" + name, [128, cap // 16], I16, kind="ExternalInput"
        )
        outs[name] = nc.dram_tensor("o_" + name, [cap], F32, kind="ExternalOutput")
    for name, cc, t in NT_SEGS:
        ins[name] = nc.dram_tensor("x_" + name, [NT_PAD], I32, kind="ExternalInput")
        outs[name] = nc.dram_tensor(
            "o_" + name, [NT_PAD * S], F32, kind="ExternalOutput"
        )

    with ExitStack() as ctx:
        tc = ctx.enter_context(TileContext(nc))
        sing = ctx.enter_context(tc.tile_pool(name="sing", bufs=1))
        gb = ctx.enter_context(tc.tile_pool(name="gbuf", bufs=4))
        tmpp = ctx.enter_context(tc.tile_pool(name="tmp", bufs=2))
        ib = ctx.enter_context(tc.tile_pool(name="ibuf", bufs=2))
        scp = ctx.enter_context(tc.tile_pool(name="sc", bufs=3))
        pp1 = ctx.enter_context(tc.tile_pool(name="pp1", bufs=1, space="PSUM"))
        ppt = ctx.enter_context(tc.tile_pool(name="ppt", bufs=2, space="PSUM"))

        ident = sing.tile([128, 128], F32)
        make_identity(nc, ident[:, :])
        ones8 = sing.tile([8, 1], F32)
        nc.vector.memset(ones8, 1.0)
        ones1 = sing.tile([1, 128], F32)
        nc.vector.memset(ones1, 1.0)

        # ---- head: sT[d,g] = sum_k partials[k,g,d]
        pt = sing.tile([8, 6 * D], F32)
        nc.sync.dma_start(out=pt[:, :], in_=partials[:, :])
        sT_ps = pp1.tile([128, 6], F32, tag="sT")
        for g in range(6):
            nc.tensor.matmul(
                out=sT_ps[:, g : g + 1],
                lhsT=pt[:, g * D : (g + 1) * D],
                rhs=ones8[:, :],
                start=True,
                stop=True,
            )
        sT = sing.tile([128, 6], F32)
        nc.vector.tensor_copy(sT[:, :], sT_ps[:, :])

        KT = []
        for m in range(2):
            kin = sing.tile([128, 128], F32, tag=f"kin{m}")
            nc.sync.dma_start(out=kin[:, :], in_=rel[m, :, :])
            kt_ps = ppt.tile([128, 128], F32, tag="ktp")
            nc.tensor.transpose(out=kt_ps[:, :], in_=kin[:, :], identity=ident[:, :])
            kt = sing.tile([128, 128], F32, tag=f"kt{m}")
            nc.vector.tensor_copy(kt[:, :], kt_ps[:, :])
            KT.append(kt)

        # c_g = K_{g%2} @ s_g  -> [128(d), 6]
        c_ps = pp1.tile([128, 6], F32, tag="c")
        for g in range(6):
            nc.tensor.matmul(
                out=c_ps[:, g : g + 1],
                lhsT=KT[g % 2][:, :],
                rhs=sT[:, g : g + 1],
                start=True,
                stop=True,
            )
        c_sb = sing.tile([128, 6], F32)
        nc.vector.tensor_copy(c_sb[:, 0:2], c_ps[:, 0:2])
        nc.vector.tensor_scalar_mul(c_sb[:, 2:4], c_ps[:, 2:4], float(S))
        nc.vector.tensor_copy(c_sb[:, 4:6], c_ps[:, 4:6])
        # broadcast each c column to all partitions:
        # cT_g = transpose(c[:, g]) -> [1,128] at partition 0, then
        # bc_g = ones1^T @ cT_g -> [128, 128]
        CB = []
        for g in range(6):
            ct_ps = ppt.tile([1, 128], F32, tag="ctp")
            nc.tensor.transpose(
                out=ct_ps[:, :], in_=c_sb[:, g : g + 1], identity=ident[:, :]
            )
            ct1 = sing.tile([1, 128], F32, tag=f"ct{g}")
            nc.vector.tensor_copy(ct1[:, :], ct_ps[:, :])
            cb_ps = ppt.tile([128, 128], F32, tag="cbp")
            nc.tensor.matmul(
                out=cb_ps[:, :],
                lhsT=ones1[:, :],
                rhs=ct1[:, :],
                start=True,
                stop=True,
            )
            cb = sing.tile([128, 128], F32, tag=f"cb{g}")
            nc.vector.tensor_copy(cb[:, :], cb_ps[:, :])
            CB.append(cb)

        # ---- bucketed segments: gather rows, dot with c, store chunk scores
        for name, L, t, cc, chunks in L2_SEGS:
            tab = tabA if t == "A" else tabB
            nsh = _nsh(t)
            W = _cap(chunks) * nsh // 16
            it = ib.tile([128, W], I16, tag="idx")
            nc.sync.dma_start(out=it[:, :], in_=ins[name][:, :])
            col = 0
            base = 0
            for s in range(nsh):
                for n in chunks:
                    wc = n // 16
                    bt = gb.tile([128, 4096], F32, tag="g")
                    nc.gpsimd.dma_gather(
                        out_ap=bt[:, :n].rearrange("p (c e) -> p c e", e=D),
                        in_ap=tab[s],
                        idxs_ap=it[:, col : col + wc],
                        num_idxs=n,
                        num_idxs_reg=n,
                        elem_size=D,
                        single_packet=False,
                    )
                    tmp = tmpp.tile([128, 4096], F32, tag="t")
                    bc = CB[cc][:, :]
                    bc_ap = bass.AP(
                        tensor=bc.tensor,
                        offset=bc.offset,
                        ap=[bc.ap[0], [0, n // 128], [1, 128]],
                    )
                    nc.vector.tensor_tensor(
                        out=tmp[:, :n],
                        in0=bt[:, :n],
                        in1=bc_ap,
                        op=mybir.AluOpType.mult,
                    )
                    sc = scp.tile([128, 32], F32, tag="s")
                    nc.vector.tensor_reduce(
                        out=sc[:, : n // 128],
                        in_=tmp[:, :n].rearrange("p (c d) -> p c d", d=D),
                        axis=X,
                        op=ADD,
                    )
                    nc.sync.dma_start(
                        out=outs[name][base : base + n].rearrange(
                            "(p c) -> p c", p=128
                        ),
                        in_=sc[:, : n // 128],
                    )
                    col += wc
                    base += n
        # ---- nt segments: order-preserving indirect gathers + x16 expand
        for name, cc, t in NT_SEGS:
            tab = tabA if t == "A" else tabB
            flat = tab[:].rearrange("s n d -> (s n) d")
            it2 = ib.tile([128, NT_W], I32, tag="idxo")
            nc.sync.dma_start(
                out=it2[:, :],
                in_=ins[name][:].rearrange("(p j) -> p j", p=128),
            )
            scn = scp.tile([128, NT_W], F32, tag="nt")
            for j in range(NT_W):
                bt2 = gb.tile([128, D], F32, tag="go")
                nc.gpsimd.indirect_dma_start(
                    out=bt2[:, :],
                    out_offset=None,
                    in_=flat,
                    in_offset=IndirectOffsetOnAxis(ap=it2[:, j : j + 1], axis=0),
                )
                tmp2 = tmpp.tile([128, D], F32, tag="t2")
                nc.vector.tensor_mul(tmp2[:, :], bt2[:, :], CB[cc][:, :])
                nc.vector.tensor_reduce(
                    out=scn[:, j : j + 1], in_=tmp2[:, :], axis=X, op=ADD
                )
            xt = scp.tile([128, NT_W * S], F32, tag="xt")
            s_ap = scn[:, :]
            bcast = bass.AP(
                tensor=s_ap.tensor,
                offset=s_ap.offset,
                ap=[s_ap.ap[0], s_ap.ap[1], [0, S]],
            )
            nc.vector.tensor_copy(xt[:, :], bcast)
            nc.sync.dma_start(
                out=outs[name][:].rearrange("(p j) -> p j", p=128), in_=xt[:, :]
            )
    nc.compile()
    return nc


_CACHE = {}


def _programs():
    if "p" not in _CACHE:
        _CACHE["p"] = (build_l1(), build_l2())
    return _CACHE["p"]


# ---------------------------------------------------------------- host glue


def _shard_tables(emb_A, emb_B):
    tabA = np.zeros((NSH_A, SHP, D), np.float32)
    for s in range(NSH_A):
        tabA[s, :SH] = emb_A[s * SH : (s + 1) * SH]
    tabB = np.zeros((NSH_B, SHP, D), np.float32)
    for s in range(NSH_B):
        tabB[s, :SH] = emb_B[s * SH : (s + 1) * SH]
    return tabA, tabB


def _wrap16(stream):
    """[L] int -> [128, L//16] int16 (wrapped in 16 partitions, replicated)."""
    L = stream.shape[0]
    w = stream.reshape(L // 16, 16).T.astype(np.int16)  # [16, L/16]
    return np.tile(w, (8, 1))


def _bucketize(idx, nsh, cap):
    """Bucket by shard, pad each bucket to cap with ZIDX.

    Returns (stream [nsh*cap] local indices, qpos [len(idx)]: stream position
    of each original element)."""
    L = idx.shape[0]
    stream = np.full(nsh * cap, ZIDX, np.int64)
    qpos = np.empty(L, np.int64)
    for s in range(nsh):
        m = (idx >= s * SH) & (idx < (s + 1) * SH)
        cnt = int(m.sum())
        assert cnt <= cap, f"bucket overflow: {cnt} > {cap}"
        stream[s * cap : s * cap + cnt] = idx[m] - s * SH
        qpos[m] = s * cap + np.arange(cnt)
    return stream, qpos


def _chunk_pos_map(chunks, nsh):
    """Stream position q -> stored DRAM position.

    The device stores each chunk of size n as [128, n/128] tiles written
    p-major (DRAM[base + p*(n/128) + c]), where in-chunk index i = c*128+p."""
    cap = _cap(chunks)
    pos = np.empty(nsh * cap, np.int64)
    base = 0
    qb = 0
    for s in range(nsh):
        for n in chunks:
            i = np.arange(n)
            pos[qb : qb + n] = base + (i % 128) * (n // 128) + i // 128
            base += n
            qb += n
    return pos


def kernel(
    emb_A,
    emb_B,
    rel_emb,
    edge_index_m0,
    edge_index_m1,
    neg_head_m0,
    neg_head_m1,
    neg_tail_m0,
    neg_tail_m1,
    _results=None,
):
    emb_A = np.ascontiguousarray(np.asarray(emb_A, dtype=np.float32))
    emb_B = np.ascontiguousarray(np.asarray(emb_B, dtype=np.float32))
    rel_emb = np.ascontiguousarray(np.asarray(rel_emb, dtype=np.float32))
    ei0 = np.asarray(edge_index_m0, dtype=np.int64)
    ei1 = np.asarray(edge_index_m1, dtype=np.int64)
    nh0 = np.asarray(neg_head_m0, dtype=np.int64)
    nh1 = np.asarray(neg_head_m1, dtype=np.int64)
    nt0 = np.asarray(neg_tail_m0, dtype=np.int64)
    nt1 = np.asarray(neg_tail_m1, dtype=np.int64)

    l1, l2 = _programs()
    cores = list(range(NCORES))
    tabA, tabB = _shard_tables(emb_A, emb_B)

    l1_idx = {
        "s0": ei0[1], "s1": ei1[1], "h0": nh0[:, 0], "h1": nh1[:, 0],
        "t0": nt0.reshape(-1), "t1": nt1.reshape(-1),
    }
    in1 = []
    for k in cores:
        m = {"tabA": tabA, "tabB": tabB}
        for name, L, t, chunks in L1_GROUPS:
            arr = l1_idx[name]
            per = arr.shape[0] // NCORES
            sl = arr[k * per : (k + 1) * per]
            stream, _ = _bucketize(sl, _nsh(t), _cap(chunks))
            m["x_" + name] = _wrap16(stream)
        in1.append(m)
    r1 = run_bass_kernel_spmd(l1, in1, cores)
    partials = np.stack([r1.results[k]["partials"].reshape(6 * D) for k in cores])

    l2_idx = {
        "pos0": ei0[0], "pos1": ei1[0],
        "nh0": nh0.reshape(-1), "nh1": nh1.reshape(-1),
        "nt0": nt0[:, 0], "nt1": nt1[:, 0],
    }
    in2 = []
    take_maps = []  # per core, per segment: dram positions in original order
    for k in cores:
        m = {"tabA": tabA, "tabB": tabB, "partials": partials, "rel": rel_emb}
        tm = {}
        for name, L, t, cc, chunks in L2_SEGS:
            arr = l2_idx[name]
            per = arr.shape[0] // NCORES
            sl = arr[k * per : (k + 1) * per]
            stream, qpos = _bucketize(sl, _nsh(t), _cap(chunks))
            m["x_" + name] = _wrap16(stream)
            tm[name] = _chunk_pos_map(chunks, _nsh(t))[qpos]
        for name, cc, t in NT_SEGS:
            arr = l2_idx[name]
            per = arr.shape[0] // NCORES
            sl = arr[k * per : (k + 1) * per]
            # global row -> padded-shard-flat row (tab viewed [(s n) d])
            adj = (sl // SH) * SHP + (sl % SH)
            m["x_" + name] = np.concatenate(
                [adj, np.zeros(NT_PAD - per, np.int64)]
            ).astype(np.int32)
        take_maps.append(tm)
        in2.append(m)
    r2 = run_bass_kernel_spmd(l2, in2, cores)

    segs = {}
    for name, L, t, cc, chunks in L2_SEGS:
        segs[name] = np.concatenate(
            [r2.results[k]["o_" + name][take_maps[k][name]] for k in cores]
        )
    for name, cc, t in NT_SEGS:
        # o flat position (p*NT_W + j)*S + s_ = e_local*S + s_ : contiguous
        segs[name] = np.concatenate(
            [r2.results[k]["o_" + name][: EC * S] for k in cores]
        )
    if _results is not None:
        _results.extend([r1, r2])
    return np.concatenate(
        [segs["pos0"], segs["pos1"], segs["nh0"], segs["nh1"],
         segs["nt0"], segs["nt1"]]
    )
